# revision 1
# baseline (speedup 1.0000x reference)
"""8-NeuronCore Trainium2 kernel for nn_AttentionBlock_17789754540111.

Strategy (per the sharding hint): data-parallel over the spatial H axis —
each of the 8 cores owns H/8 = 4 rows of the 32x32 spatial grid for all
(T, B), with parameters replicated. The attention batch dim is (B, H, W),
so attention (over T) is fully core-local. The only cross-core coupling is
the two InstanceNorms, whose (H, W) statistics are formed from per-core
partial sums combined with an 8-way on-device AllReduce (jax.lax.psum).

The module is compiled via the Neuron compiler into a single SPMD NEFF per
core (one fused device program: norm -> QKV -> attention -> norm -> proj ->
residual); host work is only the shard/unshard of the I/O tensors.
"""

import math

import numpy as np
import jax
import jax.numpy as jnp
from jax.experimental.shard_map import shard_map
from jax.sharding import Mesh, PartitionSpec as P

T, B, H, W, C = 64, 2, 32, 32, 128
HE = 8
HD = C // HE
EPS_IN = 1e-5
EPS_LN = 1e-5
NUM_BUCKETS = 32
MAX_DIST = 32
NCORES = 8

# Static T5-style bucket table (input-independent).
_rp = np.arange(T)[None, :] - np.arange(T)[:, None]
_n = -_rp
_ret = (_n < 0).astype(np.int64) * (NUM_BUCKETS // 2)
_n = np.abs(_n)
_mx = NUM_BUCKETS // 4
_is_small = _n < _mx
_vl = _mx + (
    np.log(np.maximum(_n, 1).astype(np.float64) / _mx)
    / math.log(MAX_DIST / _mx)
    * (NUM_BUCKETS // 2 - _mx)
).astype(np.int64)
_vl = np.minimum(_vl, NUM_BUCKETS // 2 - 1)
_BUCKETS = (_ret + np.where(_is_small, _n, _vl)).astype(np.int32)  # (T, T)

_COMPILED = None


def _core_fn(x, norm1_w, norm1_b, w_in, b_in, qn_w, qn_b, kn_w, kn_b,
             rel_emb, norm2_w, norm2_b, w_out, b_out, gamma):
    """Runs on ONE NeuronCore. x: (T, B, HL, W, C) local H-shard."""
    t, b, hl, w_, c = x.shape
    n_spatial = H * W  # global count for the instance-norm denominators

    def inorm_global(v, wt, bs):
        # v: (t*b, hl, w, c); stats over the FULL (H, W) extent via psum.
        s1 = jnp.sum(v, axis=(1, 2), keepdims=True)
        s2 = jnp.sum(v * v, axis=(1, 2), keepdims=True)
        s1 = jax.lax.psum(s1, "x")
        s2 = jax.lax.psum(s2, "x")
        mean = s1 / n_spatial
        var = s2 / n_spatial - mean * mean
        return (v - mean) * jax.lax.rsqrt(var + EPS_IN) * wt + bs

    residual = x
    bf = jnp.bfloat16
    xf = x.reshape(t * b, hl, w_, c)
    xf = inorm_global(xf, norm1_w, norm1_b)
    # 1x1 convs in bf16 with fp32 accumulation (branch is gamma-scaled, so
    # bf16 rounding is far below the output's fp32 envelope).
    xf = jnp.einsum("nhwc,oc->nhwo", xf.astype(bf), w_in.astype(bf),
                    preferred_element_type=jnp.float32) + b_in

    xt = xf.reshape(t, b, hl, w_, HE, 3 * HD)
    xt = xt.transpose(1, 2, 3, 4, 0, 5).reshape(b * hl * w_, HE, t, 3 * HD)
    q, k, v = jnp.split(xt, 3, axis=-1)

    def lnorm(u, wt, bs):
        m = jnp.mean(u, axis=-1, keepdims=True)
        var = jnp.var(u, axis=-1, keepdims=True)
        return (u - m) * jax.lax.rsqrt(var + EPS_LN) * wt + bs

    q = lnorm(q, qn_w, qn_b)
    k = lnorm(k, kn_w, kn_b)

    bias = rel_emb[_BUCKETS]              # (T, T, HE)
    bias = bias.transpose(2, 0, 1)[None]  # (1, HE, T, T)

    scale = HD ** -0.5
    attn = jnp.einsum("bhqd,bhkd->bhqk", q.astype(bf), k.astype(bf),
                      preferred_element_type=jnp.float32) * scale + bias
    attn = jax.nn.softmax(attn, axis=-1)
    out = jnp.einsum("bhqk,bhkd->bhqd", attn.astype(bf), v.astype(bf),
                     preferred_element_type=jnp.float32)  # (bhw, HE, t, HD)

    out = out.reshape(b, hl, w_, HE, t, HD).transpose(4, 0, 1, 2, 3, 5)
    out = out.reshape(t * b, hl, w_, c)
    out = inorm_global(out, norm2_w, norm2_b)
    out = jnp.einsum("nhwc,oc->nhwo", out.astype(bf), w_out.astype(bf),
                     preferred_element_type=jnp.float32) + b_out
    out = out.reshape(t, b, hl, w_, c)
    return residual + out * gamma


def _get_compiled():
    global _COMPILED
    if _COMPILED is None:
        devs = jax.devices()[:NCORES]
        assert len(devs) == NCORES, f"need {NCORES} cores, got {len(devs)}"
        mesh = Mesh(np.array(devs), ("x",))
        x_spec = P(None, None, "x", None, None)   # shard H
        rep = P()
        in_specs = (x_spec,) + (rep,) * 14
        fn = shard_map(_core_fn, mesh=mesh, in_specs=in_specs,
                       out_specs=x_spec, check_rep=False)
        _COMPILED = jax.jit(fn)
    return _COMPILED


def kernel(**inputs) -> np.ndarray:
    order = ["x", "norm1_w", "norm1_b", "w_in", "b_in", "qn_w", "qn_b",
             "kn_w", "kn_b", "rel_emb", "norm2_w", "norm2_b", "w_out",
             "b_out", "gamma"]
    args = [np.asarray(inputs[k], dtype=np.float32) for k in order]
    out = _get_compiled()(*args)
    return np.asarray(out, dtype=np.float32)


if __name__ == "__main__":
    rng = np.random.default_rng(0)
    ins = {
        "x": rng.standard_normal((T, B, H, W, C), dtype=np.float32),
        "norm1_w": np.ones(C, np.float32), "norm1_b": np.zeros(C, np.float32),
        "w_in": rng.standard_normal((3 * C, C)).astype(np.float32) * 0.02,
        "b_in": np.zeros(3 * C, np.float32),
        "qn_w": np.ones(HD, np.float32), "qn_b": np.zeros(HD, np.float32),
        "kn_w": np.ones(HD, np.float32), "kn_b": np.zeros(HD, np.float32),
        "rel_emb": rng.standard_normal((NUM_BUCKETS, HE)).astype(np.float32) * 0.02,
        "norm2_w": np.ones(C, np.float32), "norm2_b": np.zeros(C, np.float32),
        "w_out": rng.standard_normal((C, C)).astype(np.float32) * 0.02,
        "b_out": np.zeros(C, np.float32),
        "gamma": np.full(C, 1e-6, np.float32),
    }
    y = kernel(**ins)
    print("kernel ran, out shape", y.shape, y.dtype)



# revision 2
# speedup vs baseline: 3.5325x; 3.5325x over previous
"""8-NeuronCore Trainium2 Bass kernel for nn_AttentionBlock_17789754540111.

Self-contained: builds a hand-written Bass/Tile SPMD program (H-sharded over
8 cores, params replicated, instance-norm stats combined via on-device
AllReduce) and runs it on the 8 axon-tunneled TRN2 cores via
concourse.bass_utils.run_bass_kernel_spmd.
"""

import sys
import types
import ctypes
import contextlib

# ---- NTFF profile hook expected by bass_utils under axon ----------------
def _install_axon_hook():
    if "antenv.axon_hooks" in sys.modules:
        return
    hookmod = types.ModuleType("antenv.axon_hooks")

    def _make_hook():
        try:
            lib = ctypes.CDLL("/opt/axon/libaxon_pjrt.so")
        except OSError:
            return None
        if not hasattr(lib, "axon_start_nrt_profile"):
            return None
        lib.axon_start_nrt_profile.argtypes = [ctypes.POINTER(ctypes.c_int64),
                                               ctypes.c_size_t]
        lib.axon_start_nrt_profile.restype = ctypes.c_int64
        lib.axon_stop_nrt_profile.argtypes = [ctypes.c_char_p]
        lib.axon_stop_nrt_profile.restype = ctypes.c_int64

        @contextlib.contextmanager
        def _hook(output_dir, device_ids):
            import jax
            jax.devices()
            if device_ids:
                ids = (ctypes.c_int64 * len(device_ids))(*device_ids)
                rc = lib.axon_start_nrt_profile(ids, len(device_ids))
            else:
                rc = lib.axon_start_nrt_profile(None, 0)
            if rc != 0:
                raise RuntimeError(f"axon_start_nrt_profile rc={rc}")
            try:
                yield
            finally:
                lib.axon_stop_nrt_profile(str(output_dir).encode())
        return _hook

    hook = _make_hook()
    hookmod.get_axon_ntff_profile_hook = lambda: hook
    hookmod.set_axon_ntff_profile_hook = lambda h: None
    sys.modules["antenv.axon_hooks"] = hookmod

_install_axon_hook()

import math
import sys

sys.path.insert(0, "/opt/trn_rl_repo")

import numpy as np

import concourse.bass as bass  # noqa
import concourse.bacc as bacc
import concourse.mybir as mybir
from concourse import tile

F32 = mybir.dt.float32
BF16 = mybir.dt.bfloat16
AF = mybir.ActivationFunctionType
ALU = mybir.AluOpType
AX = mybir.AxisListType

T, B, H, W, C = 64, 2, 32, 32, 128
HE, HD = 8, 16
NCORES = 8
HL = H // NCORES
PIX = B * HL * W                  # 256
NP_ = T * PIX                     # 16384
NSAMP = T * B                     # 128
SPATIAL = H * W                   # 1024 (global)
EPS = 1e-5
VST = 136
NPAIR = PIX // 2                  # 128


def _rel_bias_table(rel_emb):
    rp = np.arange(T)[None, :] - np.arange(T)[:, None]
    n = -rp
    ret = (n < 0).astype(np.int64) * 16
    n = np.abs(n)
    mx = 8
    small = n < mx
    vl = mx + (np.log(np.maximum(n, 1) / mx) / math.log(32 / mx) * 8).astype(np.int64)
    vl = np.minimum(vl, 15)
    buckets = ret + np.where(small, n, vl)
    return np.ascontiguousarray(
        rel_emb[buckets].transpose(2, 0, 1)).astype(np.float32)  # (he, tq, tk)


def host_prep(inputs):
    w_in = np.asarray(inputs["w_in"], np.float32)
    b_in = np.asarray(inputs["b_in"], np.float32)
    rows = w_in.reshape(HE, 3, HD, C)
    b3 = b_in.reshape(HE, 3, HD)
    wq = rows[:, 0].reshape(HE * HD, C)
    wk = rows[:, 1].reshape(HE * HD, C)
    wv = rows[:, 2].reshape(HE * HD, C)

    bias = _rel_bias_table(np.asarray(inputs["rel_emb"], np.float32))
    eb = np.exp(bias)                                          # (he, tq, tk)
    expb, wo, n2w, n2b = {}, {}, {}, {}
    n2w_full = np.asarray(inputs["norm2_w"], np.float32)
    n2b_full = np.asarray(inputs["norm2_b"], np.float32)
    w_out = np.asarray(inputs["w_out"], np.float32)
    for eo in (0, 1):
        heads = [2 * j + eo for j in range(4)]
        h = eb[heads]                                          # (4, tq, tk)
        # col = j4*128 + jj*64 + tq ; rows = 2 px * 64 tk
        e1 = h.transpose(2, 0, 1)                              # (tk, j, tq)
        e2 = np.stack([e1, e1], axis=2).reshape(T, 4 * 2 * T)  # (tk, (j, jj, tq))
        expb[eo] = np.concatenate([e2, e2], axis=0).astype(np.float32)
        m = np.zeros((128, 128), np.float32)
        wv_ = np.zeros((128, 1), np.float32)
        bv_ = np.zeros((128, 1), np.float32)
        for j, he in enumerate(heads):
            m[32 * j: 32 * j + 16, :] = w_out[:, he * 16: (he + 1) * 16].T
            wv_[32 * j: 32 * j + 16, 0] = n2w_full[he * 16: (he + 1) * 16]
            bv_[32 * j: 32 * j + 16, 0] = n2b_full[he * 16: (he + 1) * 16]
        wo[eo], n2w[eo], n2b[eo] = m, wv_, bv_

    J = np.kron(np.eye(HE, dtype=np.float32), np.ones((HD, HD), np.float32))
    E4 = np.zeros((4, 128), np.float32)
    for s in range(4):
        E4[s, 32 * s: 32 * s + 16] = 1.0

    col = lambda a: np.asarray(a, np.float32).reshape(-1, 1)
    return {
        "ident": np.eye(128, dtype=np.float32),
        "identb": np.eye(128, dtype=np.float32),
        "wqT": wq.T.copy(), "wkT": wk.T.copy(), "wvT": wv.T.copy(),
        "bq": col(b3[:, 0].reshape(-1)), "bk": col(b3[:, 1].reshape(-1)),
        "bvrow": np.broadcast_to(b3[:, 2].reshape(-1), (128, 128)).copy(),
        "expbE": expb[0], "expbO": expb[1],
        "J": J, "E4": E4, "woE": wo[0], "woO": wo[1],
        "b_o": col(inputs["b_out"]), "gamma": col(inputs["gamma"]),
        "n1w": col(inputs["norm1_w"]), "n1b": col(inputs["norm1_b"]),
        "epscol": np.full((128, 1), EPS, np.float32),
        "n2wE": n2w[0], "n2bE": n2b[0], "n2wO": n2w[1], "n2bO": n2b[1],
        "qnw": col(np.tile(np.asarray(inputs["qn_w"], np.float32), HE)),
        "qnb": col(np.tile(np.asarray(inputs["qn_b"], np.float32), HE)),
        "knw": col(np.tile(np.asarray(inputs["kn_w"], np.float32), HE)),
        "knb": col(np.tile(np.asarray(inputs["kn_b"], np.float32), HE)),
    }


_BF16_IN = {"wqT", "wkT", "wvT", "bvrow", "expbE", "expbO", "J", "E4",
            "woE", "woO", "identb"}
_CONST_SHAPES = {
    "ident": (128, 128), "identb": (128, 128), "wqT": (128, 128), "wkT": (128, 128), "wvT": (128, 128),
    "bq": (128, 1), "bk": (128, 1), "bvrow": (128, 128),
    "expbE": (128, 512), "expbO": (128, 512), "J": (128, 128), "E4": (4, 128),
    "woE": (128, 128), "woO": (128, 128), "b_o": (128, 1), "gamma": (128, 1),
    "n1w": (128, 1), "n1b": (128, 1), "epscol": (128, 1),
    "n2wE": (128, 1), "n2bE": (128, 1), "n2wO": (128, 1), "n2bO": (128, 1),
    "qnw": (128, 1), "qnb": (128, 1), "knw": (128, 1), "knb": (128, 1),
}


def build_nc(ln_general=True):
    nc = bacc.Bacc("TRN2", target_bir_lowering=False, debug=False,
                   enable_asserts=False, num_devices=NCORES)
    ins = {"x": nc.dram_tensor("x", [NP_, C], F32, kind="ExternalInput").ap()}
    for name, shp in _CONST_SHAPES.items():
        dt = BF16 if name in _BF16_IN else F32
        ins[name] = nc.dram_tensor(name, list(shp), dt, kind="ExternalInput").ap()
    y = nc.dram_tensor("y", [NP_, C], F32, kind="ExternalOutput").ap()
    with tile.TileContext(nc) as tc:
        _body(tc, nc, ins, y, ln_general)
    nc.compile()
    return nc


def _body(tc, nc, ins, y, ln_general):
    x = ins["x"]
    pools = {}

    def pool(name, bufs=1, space="SBUF"):
        if name not in pools:
            pools[name] = tc.alloc_tile_pool(name=name, bufs=bufs, space=space)
        return pools[name]

    cp = pool("consts")
    big = pool("big")
    ps = pool("psA", bufs=2, space="PSUM")
    ps_s = pool("psS", bufs=1, space="PSUM")
    ps_t = pool("psT", bufs=1, space="PSUM")
    dram = pool("dram", bufs=1, space="DRAM")
    sm = pool("small", bufs=1)
    ldp = pool("ldp", bufs=2)
    lnp = pool("lnp", bufs=1)
    esp = pool("esp", bufs=2)
    outp = pool("outp", bufs=2)

    cst = {}
    for name in _CONST_SHAPES:
        ap = ins[name]
        t = cp.tile(list(ap.shape), ap.dtype, tag=name)
        nc.sync.dma_start(t[:], ap)
        cst[name] = t

    # Big slots: S1 {xT, v_t} / S2 {kk} / S3 {xn, attH} / S4 {q} / S5 {sq, kEO}
    xT = big.tile([128, VST * NPAIR + 32], BF16, tag="S1", name="xT")[:, :NP_]
    q = big.tile([128, NP_], BF16, tag="S4")
    kk = big.tile([128, NP_], BF16, tag="S2")

    # ---------------- Phase 0: load + transpose --------------------------
    for k8 in range(32):
        ld = ldp.tile([128, 512], F32, tag="ld")
        src = x.rearrange("(k8 k p) c -> k8 p k c", p=128, k=4)[k8]
        nc.sync.dma_start(ld[:].rearrange("p (k c) -> p k c", k=4), src)
        for j in range(4):
            k = 4 * k8 + j
            pt = ps_t.tile([128, 128], F32, tag="t", name="tp")
            nc.tensor.transpose(pt[:], ld[:, 128 * j: 128 * (j + 1)], cst["ident"][:])
            if j % 2 == 0:
                nc.vector.tensor_copy(xT[:, 128 * k: 128 * (k + 1)], pt[:])
            else:
                nc.scalar.copy(xT[:, 128 * k: 128 * (k + 1)], pt[:])

    # ---------------- Phase 1: norm1 stats + AllReduce -------------------
    s1 = sm.tile([128, NSAMP], F32, tag="s1")
    s2 = sm.tile([128, NSAMP], F32, tag="s2")
    sq = big.tile([128, NP_], BF16, tag="S5")
    nc.vector.reduce_sum(s1[:], xT[:].rearrange("p (k s) -> p k s", s=128), axis=AX.X)
    nc.scalar.square(sq[:], xT[:])
    nc.vector.reduce_sum(s2[:], sq[:].rearrange("p (k s) -> p k s", s=128), axis=AX.X)

    stats = sm.tile([128, 2 * NSAMP], F32, tag="st2")
    nc.vector.tensor_copy(stats[:, :NSAMP], s1[:])
    nc.vector.tensor_copy(stats[:, NSAMP:], s2[:])
    cc_in = dram.tile([128, 2 * NSAMP], F32, tag="cc_in")
    cc_out = dram.tile([128, 2 * NSAMP], F32, tag="cc_out")
    nc.gpsimd.dma_start(cc_in[:], stats[:])
    nc.gpsimd.collective_compute("AllReduce", ALU.add,
                                 replica_groups=[list(range(NCORES))],
                                 ins=[cc_in[:].opt()], outs=[cc_out[:].opt()])
    nc.sync.dma_start(stats[:], cc_out[:])

    mean = sm.tile([128, NSAMP], F32, tag="m2")
    scl1 = sm.tile([128, NSAMP], F32, tag="sc2")
    sft1 = sm.tile([128, NSAMP], F32, tag="sf2")
    tmp = sm.tile([128, NSAMP], F32, tag="t2")

    def norm_coeffs(st, scl, sft, mn, tp, w_ap, b_ap):
        nc.vector.tensor_scalar_mul(mn[:], st[:, :NSAMP], 1.0 / SPATIAL)
        nc.vector.tensor_scalar_mul(tp[:], st[:, NSAMP:], 1.0 / SPATIAL)
        nc.vector.tensor_tensor(out=scl[:], in0=mn[:], in1=mn[:], op=ALU.mult)
        nc.vector.tensor_tensor(out=tp[:], in0=tp[:], in1=scl[:], op=ALU.subtract)
        nc.scalar.activation(tp[:], tp[:], AF.Sqrt, bias=cst["epscol"][:, 0:1], scale=1.0)
        nc.vector.reciprocal(scl[:], tp[:])
        nc.vector.tensor_scalar_mul(scl[:], scl[:], w_ap[:, 0:1])
        nc.vector.tensor_tensor(out=sft[:], in0=mn[:], in1=scl[:], op=ALU.mult)
        nc.vector.tensor_scalar(out=sft[:], in0=sft[:], scalar1=-1.0,
                                scalar2=b_ap[:, 0:1], op0=ALU.mult, op1=ALU.add)

    norm_coeffs(stats, scl1, sft1, mean, tmp, cst["n1w"], cst["n1b"])

    # ---------------- Phase 2: norm1 apply -------------------------------
    xn = big.tile([128, NP_], BF16, tag="S3")
    for k in range(NSAMP):
        sl = slice(128 * k, 128 * (k + 1))
        nc.vector.tensor_scalar(out=xn[:, sl], in0=xT[:, sl],
                                scalar1=scl1[:, k: k + 1], scalar2=sft1[:, k: k + 1],
                                op0=ALU.mult, op1=ALU.add)

    # ---------------- Phase 3: conv1 q, k --------------------------------
    for n in range(32):
        sl = slice(512 * n, 512 * (n + 1))
        pq = ps.tile([128, 512], F32, tag="a", name="pq")
        nc.tensor.matmul(pq[:], cst["wqT"][:], xn[:, sl], start=True, stop=True)
        nc.vector.tensor_scalar_add(q[:, sl], pq[:], cst["bq"][:, 0:1])
        pk = ps_s.tile([128, 512], F32, tag="s", name="pk")
        nc.tensor.matmul(pk[:], cst["wkT"][:], xn[:, sl], start=True, stop=True)
        nc.scalar.add(kk[:, sl], pk[:], cst["bk"][:, 0:1])

    # ---------------- Phase 4: LN on q and k (chunked) -------------------
    def layernorm(tt, w_ap, b_ap):
        for n in range(16):
            base = 1024 * n
            mu = lnp.tile([128, 1024], BF16, tag="ln_mu")
            vv = lnp.tile([128, 1024], BF16, tag="ln_vv")
            t1 = lnp.tile([128, 1024], BF16, tag="ln_t1")
            for c4 in range(2):
                sl = slice(base + 512 * c4, base + 512 * (c4 + 1))
                sll = slice(512 * c4, 512 * (c4 + 1))
                nc.scalar.square(t1[:, sll], tt[:, sl])
                pm = ps.tile([128, 512], F32, tag="a", name="pm")
                nc.tensor.matmul(pm[:], cst["J"][:], tt[:, sl], start=True, stop=True)
                pv = ps_s.tile([128, 512], F32, tag="s", name="pv")
                nc.tensor.matmul(pv[:], cst["J"][:], t1[:, sll], start=True, stop=True)
                nc.scalar.mul(mu[:, sll], pm[:], 1.0 / HD)
                nc.scalar.activation(vv[:, sll], pv[:], AF.Copy, bias=0.0,
                                     scale=1.0 / HD)
            nc.vector.tensor_tensor(out=t1[:], in0=mu[:], in1=mu[:], op=ALU.mult)
            nc.vector.tensor_tensor(out=vv[:], in0=vv[:], in1=t1[:], op=ALU.subtract)
            nc.scalar.activation(vv[:], vv[:], AF.Ln, bias=cst["epscol"][:, 0:1], scale=1.0)
            nc.scalar.activation(vv[:], vv[:], AF.Exp, bias=0.0, scale=-0.5)
            sl = slice(base, base + 1024)
            nc.vector.tensor_tensor(out=t1[:], in0=tt[:, sl], in1=mu[:],
                                    op=ALU.subtract)
            nc.vector.tensor_tensor(out=tt[:, sl], in0=t1[:], in1=vv[:], op=ALU.mult)
            if ln_general:
                nc.vector.tensor_scalar(out=tt[:, sl], in0=tt[:, sl],
                                        scalar1=w_ap[:, 0:1], scalar2=b_ap[:, 0:1],
                                        op0=ALU.mult, op1=ALU.add)

    layernorm(q, cst["qnw"], cst["qnb"])
    layernorm(kk, cst["knw"], cst["knb"])

    # ---------------- Phase 5: conv v (transposed layout) ----------------
    v_t = big.tile([128, VST * NPAIR + 32], BF16, tag="S1")
    nc.vector.memset(v_t[:], 0.0)
    ones_ap = v_t[:, :VST * NPAIR].rearrange(
        "p (m e d) -> p m e d", e=8, d=17)[:, :, :, 16:17]
    nc.vector.memset(ones_ap, 1.0)
    xnv = xn[:].rearrange("c (t m z) -> c t m z", m=NPAIR, z=2)
    for m in range(NPAIR):
        pv = ps_t.tile([128, 128], F32, tag="t", name="pvt")
        for z in range(2):
            nc.tensor.matmul(pv[64 * z: 64 * z + 64, :], xnv[:, :, m, z],
                             cst["wvT"][:], start=True, stop=True,
                             tile_position=(0, 64 * z))
        dst = v_t[:, VST * m: VST * m + 136].rearrange(
            "p (e d) -> p e d", d=17)[:, :, 0:16]
        nc.vector.tensor_tensor(out=dst,
                                in0=pv[:].rearrange("p (e d) -> p e d", d=16),
                                in1=cst["bvrow"][:].rearrange("p (e d) -> p e d", d=16),
                                op=ALU.add)

    # ---------------- Phases 6-8: attention (two head parities) ----------
    attH = big.tile([128, NP_], BF16, tag="S3")
    attE_d = dram.tile([128, NP_], BF16, tag="attE_d")
    kE_d = dram.tile([128, NP_], BF16, tag="kE_d")
    kO_d = dram.tile([128, NP_], BF16, tag="kO_d")
    rs_d = dram.tile([4, NP_], F32, tag="rs_d")
    qv = q[:].rearrange("a (t p) -> a t p", p=256)
    kEO = big.tile([128, NP_], BF16, tag="S5")
    kv = kEO[:].rearrange("a (t p) -> a t p", p=256)
    # Build zero-padded parity copies of k in DRAM (DMA-only: engine ops
    # cannot address 16-row partition groups; SBUF-strided DMA APs defeat
    # the race tracker, so stride only on the DRAM side).
    k_d = dram.tile([128, NP_], BF16, tag="k_d")
    nc.vector.memset(kEO[:], 0.0)
    nc.sync.dma_start(k_d[:], kk[:])
    nc.sync.dma_start(kE_d[:], kEO[:])
    nc.sync.dma_start(kO_d[:], kEO[:])
    for par, kd in ((0, kE_d), (1, kO_d)):
        src = k_d[:].rearrange("(s r) n -> s r n", r=32)[:, 16 * par: 16 * par + 16, :]
        dst = kd[:].rearrange("(s r) n -> s r n", r=32)[:, 16 * par: 16 * par + 16, :]
        nc.sync.dma_start(dst, src)

    def att_pass(eo, expb_c, woname, wname, bname):
        # build kEO for this parity
        nc.sync.dma_start(kEO[:], (kE_d if eo == 0 else kO_d)[:])

        for grp in range(32):
            pa0 = ps.tile([128, 256], F32, tag="pa0", name="pa0", bufs=1)
            pa1 = ps.tile([128, 256], F32, tag="pa1", name="pa1", bufs=1)
            for sg in range(2):
                pairs = (8 * grp + 4 * sg, 8 * grp + 4 * sg + 2)
                es = esp.tile([128, 512], BF16, tag="es")
                for rr in range(2):
                    qk0 = ps_s.tile([128, 128], F32, tag="qk0", name="qk0", bufs=1)
                    qk1 = ps_s.tile([128, 128], F32, tag="qk1", name="qk1", bufs=1)
                    for rb, qkb in ((0, qk0), (1, qk1)):
                        r = 2 * rr + rb
                        prt = slice(32 * r, 32 * r + 32)
                        for jj, pbase in enumerate(pairs):
                            for h01 in range(2):
                                p = pbase + h01
                                for tkc in range(2):
                                    nc.tensor.matmul(
                                        qkb[64 * h01 + 32 * tkc:
                                            64 * h01 + 32 * tkc + 32,
                                            64 * jj: 64 * jj + 64],
                                        kv[prt, 32 * tkc: 32 * tkc + 32, p],
                                        qv[prt, :, p], start=True, stop=True,
                                        tile_position=(32 * r,
                                                       64 * h01 + 32 * tkc))
                        nc.scalar.activation(
                            es[:, 128 * r: 128 * r + 128], qkb[:], AF.Exp,
                            bias=0.0, scale=0.25)
                nc.vector.tensor_tensor(out=es[:], in0=es[:], in1=expb_c[:],
                                        op=ALU.mult)
                for jj, pbase in enumerate(pairs):
                    pair = pbase // 2
                    lp = 2 * sg + jj
                    for h01, pah in ((0, pa0), (1, pa1)):
                        for j4 in range(4):
                            eg = 2 * j4 + eo
                            lhs_v = v_t[64 * h01: 64 * h01 + 64,
                                        VST * pair + 17 * eg: VST * pair + 17 * eg + 32]
                            nc.tensor.matmul(
                                pah[32 * j4: 32 * j4 + 32, 64 * lp: 64 * lp + 64],
                                lhs_v,
                                es[64 * h01: 64 * h01 + 64,
                                   128 * j4 + 64 * jj: 128 * j4 + 64 * jj + 64],
                                start=True, stop=True,
                                tile_position=(64 * h01, 32 * j4))
            for h01, pah in ((0, pa0), (1, pa1)):
                dstv = attH[:, 512 * grp: 512 * (grp + 1)].rearrange(
                    "c (lp z) -> c lp z", z=128)[:, :, 64 * h01: 64 * h01 + 64]
                srcv = pah[:].rearrange("c (lp z) -> c lp z", z=64)
                if h01 == 0:
                    nc.vector.tensor_copy(dstv, srcv)
                else:
                    nc.scalar.copy(dstv, srcv)

        # rowsum division
        rs = sm.tile([128, 512], F32, tag="rs")
        for s in range(4):
            nc.gpsimd.dma_start(rs_d[s: s + 1, :], attH[32 * s + 16: 32 * s + 17, :])
        nc.sync.dma_start(rs[:], rs_d[:].rearrange("e (g n) -> (e g) n", n=512))
        nc.vector.reciprocal(rs[:], rs[:])
        nc.gpsimd.dma_start(rs_d[:].rearrange("e (g n) -> (e g) n", n=512), rs[:])
        for n in range(32):
            sl = slice(512 * n, 512 * (n + 1))
            rs8 = esp.tile([4, 512], BF16, tag="rs8")
            nc.gpsimd.dma_start(rs8[:], rs_d[:, sl])
            pb = ps_s.tile([128, 512], F32, tag="s", name="pb")
            nc.tensor.matmul(pb[:], cst["E4"][:], rs8[:], start=True, stop=True)
            rbc = esp.tile([128, 512], BF16, tag="rbc")
            nc.scalar.copy(rbc[:], pb[:])
            nc.vector.tensor_tensor(out=attH[:, sl], in0=attH[:, sl], in1=rbc[:],
                                    op=ALU.mult)

        # norm2 stats (chunked; cols of chunk n = pixels 8n..8n+8, b = n//16)
        s1b = sm.tile([128, NSAMP], F32, tag="s1")
        s2b = sm.tile([128, NSAMP], F32, tag="s2")
        nc.vector.memset(s1b[:], 0.0)
        nc.vector.memset(s2b[:], 0.0)
        for n in range(32):
            sl = slice(512 * n, 512 * (n + 1))
            b = n // 16
            ssl = slice(64 * b, 64 * b + 64)
            sqc = esp.tile([128, 512], BF16, tag="sqc")
            nc.scalar.square(sqc[:], attH[:, sl])
            p1 = sm.tile([128, T], F32, tag="p1")
            p2 = sm.tile([128, T], F32, tag="p2")
            nc.vector.reduce_sum(
                p1[:], attH[:, sl].rearrange("c (p t) -> c t p", p=8), axis=AX.X)
            nc.vector.reduce_sum(
                p2[:], sqc[:].rearrange("c (p t) -> c t p", p=8), axis=AX.X)
            nc.vector.tensor_tensor(out=s1b[:, ssl], in0=s1b[:, ssl], in1=p1[:],
                                    op=ALU.add)
            nc.vector.tensor_tensor(out=s2b[:, ssl], in0=s2b[:, ssl], in1=p2[:],
                                    op=ALU.add)
        st2 = sm.tile([128, 2 * NSAMP], F32, tag="st2")
        nc.vector.tensor_copy(st2[:, :NSAMP], s1b[:])
        nc.vector.tensor_copy(st2[:, NSAMP:], s2b[:])
        cc2i = dram.tile([128, 2 * NSAMP], F32, tag="cc_in")
        cc2o = dram.tile([128, 2 * NSAMP], F32, tag="cc_out")
        nc.gpsimd.dma_start(cc2i[:], st2[:])
        nc.gpsimd.collective_compute("AllReduce", ALU.add,
                                     replica_groups=[list(range(NCORES))],
                                     ins=[cc2i[:].opt()], outs=[cc2o[:].opt()])
        nc.sync.dma_start(st2[:], cc2o[:])
        sc2 = sm.tile([128, NSAMP], F32, tag="sc2")
        sf2 = sm.tile([128, NSAMP], F32, tag="sf2")
        m2 = sm.tile([128, NSAMP], F32, tag="m2")
        t2 = sm.tile([128, NSAMP], F32, tag="t2")
        norm_coeffs(st2, sc2, sf2, m2, t2, cst[wname], cst[bname])
        attv = attH[:].rearrange("c (b s t) -> c b s t", b=2, s=128)
        for b in range(2):
            for t in range(T):
                colap = attv[:, b, :, t]
                j = 64 * b + t
                nc.vector.tensor_scalar(out=colap, in0=colap,
                                        scalar1=sc2[:, j: j + 1],
                                        scalar2=sf2[:, j: j + 1],
                                        op0=ALU.mult, op1=ALU.add)
        if eo == 0:
            nc.sync.dma_start(attE_d[:], attH[:])

    att_pass(0, cst["expbE"], "woE", "n2wE", "n2bE")
    att_pass(1, cst["expbO"], "woO", "n2wO", "n2bO")

    # ---------------- Phase 9: conv2 + gamma + residual + store ----------
    yv = y.rearrange("(t m z) c -> t m z c", m=NPAIR, z=2)
    xvr = x.rearrange("(t m z) c -> t m z c", m=NPAIR, z=2)
    for n in range(32):
        sl = slice(512 * n, 512 * (n + 1))
        aA = outp.tile([128, 512], BF16, tag="aA")
        nc.sync.dma_start(aA[:], attE_d[:, sl])
        po = ps.tile([128, 512], F32, tag="a", name="po")
        nc.tensor.matmul(po[:], cst["woE"][:], aA[:], start=True, stop=False)
        nc.tensor.matmul(po[:], cst["woO"][:], attH[:, sl], start=False, stop=True)
        yb = outp.tile([128, 512], BF16, tag="yb")
        nc.vector.tensor_scalar(out=yb[:], in0=po[:], scalar1=cst["b_o"][:, 0:1],
                                scalar2=cst["gamma"][:, 0:1], op0=ALU.add, op1=ALU.mult)
        xr = outp.tile([128, 512], F32, tag="xr")
        for z in range(2):
            nc.sync.dma_start(
                xr[64 * z: 64 * z + 64, :].rearrange("t (m c) -> t m c", m=4),
                xvr[:, 4 * n: 4 * n + 4, z, :])
        y8 = outp.tile([128, 512], F32, tag="y8")
        for j in range(4):
            pt = ps_t.tile([128, 128], BF16, tag="t", name="tp2")
            nc.tensor.transpose(pt[:], yb[:, 128 * j: 128 * (j + 1)], cst["identb"][:])
            nc.vector.tensor_tensor(out=y8[:, 128 * j: 128 * (j + 1)], in0=pt[:],
                                    in1=xr[:, 128 * j: 128 * (j + 1)], op=ALU.add)
        for z in range(2):
            nc.sync.dma_start(
                yv[:, 4 * n: 4 * n + 4, z, :],
                y8[64 * z: 64 * z + 64, :].rearrange("t (m c) -> t m c", m=4))

    for p_ in reversed(list(pools.values())):
        p_.release()


# ---- public entry point -------------------------------------------------
_NC = None


def _get_nc():
    global _NC
    if _NC is None:
        _NC = build_nc(ln_general=True)
    return _NC


def kernel(**inputs) -> np.ndarray:
    import ml_dtypes
    from concourse import bass_utils

    nc = _get_nc()
    consts = host_prep(inputs)
    cmap = {}
    for name, val in consts.items():
        v = np.asarray(val, np.float32)
        if name in _BF16_IN:
            v = v.astype(ml_dtypes.bfloat16)
        cmap[name] = v
    x = np.asarray(inputs["x"], np.float32)
    in_maps = []
    for c in range(NCORES):
        m = dict(cmap)
        m["x"] = np.ascontiguousarray(
            x[:, :, HL * c: HL * (c + 1), :, :].reshape(-1, C))
        in_maps.append(m)
    res = bass_utils.run_bass_kernel_spmd(
        nc, in_maps, core_ids=list(range(NCORES)), trace=False)
    y = np.zeros((T, B, H, W, C), np.float32)
    for c in range(NCORES):
        y[:, :, HL * c: HL * (c + 1), :, :] = \
            res.results[c]["y"].reshape(T, B, HL, W, C)
    return y


def kernel_traced(**inputs):
    """Like kernel() but returns (y, per_core_exec_ns, trace_path)."""
    import ml_dtypes
    from concourse import bass_utils

    nc = _get_nc()
    consts = host_prep(inputs)
    cmap = {}
    for name, val in consts.items():
        v = np.asarray(val, np.float32)
        if name in _BF16_IN:
            v = v.astype(ml_dtypes.bfloat16)
        cmap[name] = v
    x = np.asarray(inputs["x"], np.float32)
    in_maps = []
    for c in range(NCORES):
        m = dict(cmap)
        m["x"] = np.ascontiguousarray(
            x[:, :, HL * c: HL * (c + 1), :, :].reshape(-1, C))
        in_maps.append(m)
    res = bass_utils.run_bass_kernel_spmd(
        nc, in_maps, core_ids=list(range(NCORES)),
        trace=True, trace_cores=list(range(NCORES)))
    y = np.zeros((T, B, H, W, C), np.float32)
    for c in range(NCORES):
        y[:, :, HL * c: HL * (c + 1), :, :] = \
            res.results[c]["y"].reshape(T, B, HL, W, C)
    trace_path = (res.instructions_and_trace[1]
                  if res.instructions_and_trace else None)
    return y, res.exec_time_ns, trace_path
